# revision 1
# baseline (speedup 1.0000x reference)
"""Trainium2 Bass kernel for nn_AttentionGeneMLP (gnn_message_passing).

Strategy (8 NeuronCores):
  The SNP->gene mask has exactly one nonzero per SNP column, so the masked
  linear is a sparse gather/scatter.  Host-side we convert (mw, mask) from
  dense [G,S] to a sparse block layout (a pure format/layout transform: the
  kept values are mw where mask==1, no arithmetic):
    - sort SNPs by their gene, shard SNPs by gene range: core c owns genes
      [500c, 500c+500) and exactly the SNPs mapping to them (~5000).
    - chunk schedule shared by all cores (SPMD): greedy common local-gene
      boundaries such that every core has <= 128 SNPs per chunk; each chunk
      ships a [128, W=16] window tile E holding the masked weight value at
      (snp_row, local_gene - chunk_offset) -- the chunk's slice of
      (mw*mask).T -- concatenated with the chunk's x columns [128, B].
  Device: per chunk, xs = x2 * sigmoid(sv*x2 + bv)  (attention, with the
  per-SNP scale/bias computed on device from emb/proj/ln params; only NI=4
  classes), then PSUM-accumulate g[B, off:off+W] += xs.T @ E over the NCH
  chunks (PSUM pre-zeroed by the vector engine; window offsets are
  compile-time constants shared across cores).
  This streams ~2MB/core instead of ~90MB/core for the dense mw+mask.
  - ln1 stats: per-core partial (sum, sumsq) over its 500 real genes,
    AllReduce [128,2]; each core normalizes its own block + gelu.
  - fc1 sharded by contraction block: each core computes its 512-gene
    partial of all H1=1024 outputs (4 transposes + 8 matmuls), AllReduce
    y1 [128,1024]; lnA/gelu, fc2, lnB/gelu, out projection replicated.
  Per-feature parameter vectors ship as [1,N] and are partition-broadcast
  on device by the (otherwise idle) gpsimd engine.

Host-side work is limited to layout: sparse-format conversion, slicing
shards, transposing to the partition-major device layout, dtype casts.
All model arithmetic runs on device.
"""

import numpy as np
import ml_dtypes

import concourse.bass as bass
import concourse.mybir as mybir
import concourse.tile as tile
from concourse import bacc
from concourse.bass import ts
from concourse.bass_utils import run_bass_kernel_spmd
from concourse.masks import make_identity

F32 = mybir.dt.float32
BF16 = mybir.dt.bfloat16
BFNP = ml_dtypes.bfloat16

# Problem sizes (hardcoded per task contract).
B, S, G, E, NI = 128, 40000, 4000, 16, 4
H1, H2 = 1024, 256
EPS = 1e-5
NCORES = 8
GC = G // NCORES            # 500 genes per core
GB = 512                    # gene block width (500 real + 12 pad)
GPAD = NCORES * GB          # 4096 block-padded gene width
MEGA = 16                   # s-chunks per DMA mega-tile
# concatenated per-feature vector bundle: offsets into vecs [1, VTOT]
_VSPECS = [("mb", GB), ("ln1w", GB), ("ln1b", GB), ("fc1b", H1),
           ("lnAw", H1), ("lnAb", H1), ("fc2b", H2), ("lnBw", H2),
           ("lnBb", H2), ("outw", H2), ("outb", 1)]
_VOFF = {}
_o = 0
for _n, _l in _VSPECS:
    _VOFF[_n] = (_o, _l)
    _o += _l
VTOT = _o
AF = mybir.ActivationFunctionType
ALU = mybir.AluOpType


def _mega_starts(nch):
    starts = []
    c = 0
    while c < nch:
        starts.append((c, min(MEGA, nch - c)))
        c += MEGA
    return starts


def build_bass(repeat=1, struct=None):
    """Build + compile the 8-core SPMD Bass module. Returns nc."""
    if struct is None:
        struct = _CACHE["struct"]
    nch, w, offs = struct
    cww = w + B
    nc = bacc.Bacc("TRN2", target_bir_lowering=False, debug=False,
                   num_devices=NCORES)

    def din(name, shape, dt):
        return nc.dram_tensor(name, shape, dt, kind="ExternalInput")

    # big stream (partition-major: [p, chunk, E|x2] flattened on last dims)
    combA = din("combA", [128, nch * cww], BF16)
    # attention path
    oneA = din("oneA", [128, nch * NI], BF16)
    # packed tiny attention params: [E, NI | E | 2] and [NI, 3E | 2]
    epw = din("epw", [E, NI + E + 2], F32)
    pl4 = din("pl4", [NI, 3 * E + 2], F32)
    # all per-feature vectors concatenated, broadcast on device
    vecs = din("vecs", [1, VTOT], F32)
    w1A = din("w1A", [128, 4, H1], BF16)
    w2A = din("w2A", [128, 8, H2], BF16)

    out = nc.dram_tensor("out", [B, 1], F32, kind="ExternalOutput")

    tensors = {k: v for k, v in locals().items()}
    with tile.TileContext(nc) as tc:
        _body(tc, tensors, struct, repeat)
    nc.compile()
    return nc


def _ln_gelu_vec(nc, work, x_ap, d, group, w_sb, b_sb, out_ap, tag, eps_sb):
    """out = gelu(layernorm(x) * w + b); x_ap [128, d] f32 SBUF."""
    ng = d // group
    stats = work.tile([128, ng, 6], F32, tag=f"{tag}_st")
    xg = x_ap.rearrange("p (a b) -> p a b", b=group)
    for i in range(ng):
        nc.vector.bn_stats(out=stats[:, i, :], in_=xg[:, i, :])
    mv = work.tile([128, 2], F32, tag=f"{tag}_mv")
    nc.vector.bn_aggr(out=mv[:], in_=stats[:])
    std = work.tile([128, 1], F32, tag=f"{tag}_sd")
    nc.scalar.activation(std[:], mv[:, 1:2], AF.Sqrt, bias=eps_sb[:, 0:1])
    rstd = work.tile([128, 1], F32, tag=f"{tag}_rs")
    nc.vector.reciprocal(rstd[:], std[:])
    norm = work.tile([128, d], F32, tag="norm")  # shared across calls
    nc.vector.tensor_scalar(norm[:], x_ap, mv[:, 0:1], rstd[:, 0:1],
                            op0=ALU.subtract, op1=ALU.mult)
    nc.vector.tensor_mul(norm[:], norm[:], w_sb)
    nc.vector.tensor_add(norm[:], norm[:], b_sb)
    nc.scalar.activation(out_ap, norm[:], AF.Gelu)


def _body(tc, t, struct, repeat=1):
    nch, w, offs = struct
    cww = w + B
    nc = tc.nc
    ctx_pools = []

    def pool(**kw):
        p = tc.alloc_tile_pool(**kw)
        ctx_pools.append(p)
        return p

    const = pool(name="const", bufs=1)
    work = pool(name="work", bufs=1)
    combp = pool(name="combp", bufs=3)
    sigp = pool(name="sigp", bufs=3)
    xsp = pool(name="xsp", bufs=3)
    psg = pool(name="psg", bufs=1, space="PSUM")
    pssm = pool(name="pssm", bufs=1, space="PSUM")
    dram = pool(name="dram", bufs=1, space="DRAM")

    def emit():
        # ---- constants into SBUF ----
        def load_const(name, shape, dt):
            tl = const.tile(shape, dt, tag=f"c_{name}")
            nc.sync.dma_start(tl[:], t[name][tuple(slice(None) for _ in shape)])
            return tl

        one_sb = load_const("oneA", [128, nch * NI], BF16)
        w1_sb = load_const("w1A", [128, 4, H1], BF16)
        w2_sb = load_const("w2A", [128, 8, H2], BF16)

        # one DMA + chunked gpsimd broadcasts for all per-feature vectors
        vec_sb = const.tile([128, VTOT], F32, tag="b_vecs")
        nc.sync.dma_start(vec_sb[0:1, :], t["vecs"][:, :])
        for v0 in range(0, VTOT, 1024):
            v1 = min(v0 + 1024, VTOT)
            nc.gpsimd.partition_broadcast(vec_sb[:, v0:v1], vec_sb[0:1, v0:v1])

        def vslice(name):
            o, l = _VOFF[name]
            return vec_sb[:, o:o + l]

        mb_sb = vslice("mb")
        ln1w_sb = vslice("ln1w")
        ln1b_sb = vslice("ln1b")
        fc1b_sb = vslice("fc1b")
        lnAw_sb = vslice("lnAw")
        lnAb_sb = vslice("lnAb")
        fc2b_sb = vslice("fc2b")
        lnBw_sb = vslice("lnBw")
        lnBb_sb = vslice("lnBb")
        outw_sb = vslice("outw")
        outb_sb = vslice("outb")

        ident_f = const.tile([128, 128], F32, tag="ident_f")
        make_identity(nc, ident_f[:])
        eps_sb = const.tile([128, 1], F32, tag="eps")
        nc.vector.memset(eps_sb[:], EPS)

        # ---- attention scale/bias tables (tiny, K padded to 128) ----
        # epw packs [embT | projwT | swbw] on E partitions
        epw_sb = const.tile([128, NI + E + 2], F32, tag="epw")
        nc.vector.memset(epw_sb[:], 0.0)
        nc.sync.dma_start(epw_sb[:E, :], t["epw"][:, :])
        embT_sb = epw_sb[:, 0:NI]
        projwT_sb = epw_sb[:, NI:NI + E]
        swbw_sb = epw_sb[:, NI + E:NI + E + 2]
        # pl4 packs [projb4 | lniw4 | lnib4 | sbb4] on NI partitions
        pl4_sb = load_const("pl4", [NI, 3 * E + 2], F32)
        projb4_sb = pl4_sb[:, 0:E]
        lniw4_sb = pl4_sb[:, E:2 * E]
        lnib4_sb = pl4_sb[:, 2 * E:3 * E]
        sbb4_sb = pl4_sb[:, 3 * E:3 * E + 2]

        # h4 = emb @ proj_w.T + proj_b   [NI, E]
        ps_h4 = pssm.tile([128, 128], F32, tag="ps_small", name="ps_h4")[:NI, :E]
        nc.tensor.matmul(ps_h4[:], embT_sb[:], projwT_sb[:], start=True, stop=True)
        h4 = work.tile([NI, E], F32, tag="h4")
        nc.vector.tensor_add(h4[:], ps_h4[:], projb4_sb[:])
        # ln over E (free dim), partitions = NI
        st4 = work.tile([NI, 6], F32, tag="st4")
        nc.vector.bn_stats(out=st4[:], in_=h4[:])
        mv4 = work.tile([NI, 2], F32, tag="mv4")
        nc.vector.bn_aggr(out=mv4[:], in_=st4[:])
        std4 = work.tile([NI, 1], F32, tag="std4")
        nc.scalar.activation(std4[:], mv4[:, 1:2], AF.Sqrt, bias=eps_sb[:NI, 0:1])
        rstd4 = work.tile([NI, 1], F32, tag="rstd4")
        nc.vector.reciprocal(rstd4[:], std4[:])
        nc.vector.tensor_scalar(h4[:], h4[:], mv4[:, 0:1], rstd4[:, 0:1],
                                op0=ALU.subtract, op1=ALU.mult)
        nc.vector.tensor_mul(h4[:], h4[:], lniw4_sb[:])
        nc.vector.tensor_add(h4[:], h4[:], lnib4_sb[:])
        h4g = work.tile([128, E], F32, tag="h4g")
        nc.vector.memset(h4g[:], 0.0)
        nc.scalar.activation(h4g[:NI, :], h4[:], AF.Gelu)
        # transpose h4g -> [E, NI] then tab = h4g.T.T @ [sw|bw] : [NI, 2]
        ps_t4 = pssm.tile([128, 128], F32, tag="ps_small", name="ps_t4")[:E, :]
        nc.tensor.transpose(ps_t4[:], h4g[:], ident_f[:])
        h4gT = work.tile([128, NI], F32, tag="h4gT")
        nc.vector.memset(h4gT[:], 0.0)
        nc.vector.tensor_copy(h4gT[:E, :], ps_t4[:, :NI])
        ps_tab = pssm.tile([128, 128], F32, tag="ps_small", name="ps_tab")[:NI, :2]
        nc.tensor.matmul(ps_tab[:], h4gT[:], swbw_sb[:], start=True, stop=True)
        tab = work.tile([128, 2], F32, tag="tab")
        nc.vector.memset(tab[:], 0.0)
        nc.vector.tensor_add(tab[:NI, :], ps_tab[:], sbb4_sb[:])

        # per-SNP scale/bias via host one-hot planes: sv = onehot . tab[:,0]
        # tab rows -> [1, NI] at partition 0 via PE transpose, then
        # partition-broadcast and a broadcasted multiply-reduce.
        ps_sr = pssm.tile([128, 128], F32, tag="ps_small", name="ps_sr")
        nc.tensor.transpose(ps_sr[:1, :], tab[:, 0:1], ident_f[:])
        svrow = work.tile([128, NI], F32, tag="svrow")
        # fold the *2 of attn into x2 (host supplies 2x); halve scale here
        nc.scalar.mul(svrow[0:1, :], ps_sr[0:1, 0:NI], 0.5)
        nc.gpsimd.partition_broadcast(svrow[:, :], svrow[0:1, :])
        ps_br = pssm.tile([128, 128], F32, tag="ps_small", name="ps_br")
        nc.tensor.transpose(ps_br[:1, :], tab[:, 1:2], ident_f[:])
        bvrow = work.tile([128, NI], F32, tag="bvrow")
        nc.vector.tensor_copy(bvrow[0:1, :], ps_br[0:1, 0:NI])
        nc.gpsimd.partition_broadcast(bvrow[:, :], bvrow[0:1, :])

        one3 = one_sb.rearrange("p (c i) -> p c i", i=NI)
        sv = const.tile([128, nch], F32, tag="sv")
        bv = const.tile([128, nch], F32, tag="bv")
        svtmp = work.tile([128, nch, NI], F32, tag="svtmp")
        nc.vector.tensor_mul(svtmp[:], one3,
                             svrow.unsqueeze(1).broadcast_to([128, nch, NI]))
        nc.vector.reduce_sum(sv[:], svtmp[:], axis=mybir.AxisListType.X)
        nc.vector.tensor_mul(svtmp[:], one3,
                             bvrow.unsqueeze(1).broadcast_to([128, nch, NI]))
        nc.vector.reduce_sum(bv[:], svtmp[:], axis=mybir.AxisListType.X)

        # ---- main loop: stream [E|x2] chunks, accumulate g in PSUM ----
        # windowed accumulation: PSUM pre-zeroed, matmuls accumulate into
        # their chunk's [off, off+w) column window.  The attention is
        # vectorized per mega-tile: z = x2*sv + bv with stride-0 broadcast
        # of the per-(partition, chunk) scalars over the B axis.
        g_ps = psg.tile([128, GB], F32, tag="g_ps")
        nc.vector.memset(g_ps[:], 0.0)
        combA = t["combA"]
        for (c0, k) in _mega_starts(nch):
            comb = combp.tile([128, k, cww], BF16, tag="comb")
            nc.sync.dma_start(comb[:], combA[:, c0 * cww:(c0 + k) * cww]
                              .rearrange("p (k n) -> p k n", k=k))
            xv = comb[:, :, w:cww]                      # [128, k, B]
            svb = sv[:, c0:c0 + k].unsqueeze(2).broadcast_to([128, k, B])
            bvb = bv[:, c0:c0 + k].unsqueeze(2).broadcast_to([128, k, B])
            sig = sigp.tile([128, k, B], BF16, tag="sig")
            nc.vector.tensor_mul(sig[:], xv, svb)
            nc.vector.tensor_add(sig[:], sig[:], bvb)
            nc.scalar.activation(sig[:], sig[:], AF.Sigmoid)
            xs = xsp.tile([128, k, B], BF16, tag="xs")
            nc.vector.tensor_mul(xs[:], xv, sig[:])
            for j in range(k):
                c = c0 + j
                nc.tensor.matmul(g_ps[:, offs[c]:offs[c] + w], xs[:, j, :],
                                 comb[:, j, 0:w],
                                 start=False, stop=(c == nch - 1),
                                 skip_group_check=True)

        # ---- gene block: +mb, ln1 stats partial, AllReduce stats ----
        g_sb = work.tile([128, GB], F32, tag="g_sb")
        nc.vector.tensor_add(g_sb[:], g_ps[:], mb_sb[:])
        pstat = work.tile([128, 2], F32, tag="pstat")
        nc.vector.reduce_sum(pstat[:, 0:1], g_sb[:, 0:GC],
                             axis=mybir.AxisListType.X)
        gsq = work.tile([128, GC], F32, tag="gsq")
        nc.vector.tensor_mul(gsq[:], g_sb[:, 0:GC], g_sb[:, 0:GC])
        nc.vector.reduce_sum(pstat[:, 1:2], gsq[:], axis=mybir.AxisListType.X)
        cs_in = dram.tile([128, 2], F32, tag="cs_in")
        nc.sync.dma_start(cs_in[:], pstat[:])
        cs_out = dram.tile([128, 2], F32, tag="cs_out")
        nc.gpsimd.collective_compute(
            "AllReduce", ALU.add, replica_groups=[list(range(NCORES))],
            ins=[cs_in.opt()], outs=[cs_out.opt()])
        ssum = work.tile([128, 2], F32, tag="ssum")
        nc.sync.dma_start(ssum[:], cs_out[:, :])

        mv = work.tile([128, 2], F32, tag="ln1_mv")
        # mean = s1/G ; E[x^2] = s2/G
        nc.scalar.mul(mv[:], ssum[:], 1.0 / G)
        msq = work.tile([128, 1], F32, tag="ln1_msq")
        nc.vector.tensor_mul(msq[:], mv[:, 0:1], mv[:, 0:1])
        var = work.tile([128, 1], F32, tag="ln1_var")
        nc.vector.tensor_sub(var[:], mv[:, 1:2], msq[:])
        std = work.tile([128, 1], F32, tag="ln1_sd")
        nc.scalar.activation(std[:], var[:], AF.Sqrt, bias=eps_sb[:, 0:1])
        rstd = work.tile([128, 1], F32, tag="ln1_rs")
        nc.vector.reciprocal(rstd[:], std[:])
        # normalize own 512-col block (pads have w=b=0 so they become 0)
        norm = work.tile([128, GB], F32, tag="normg")
        nc.vector.tensor_scalar(norm[:], g_sb[:], mv[:, 0:1], rstd[:, 0:1],
                                op0=ALU.subtract, op1=ALU.mult)
        nc.vector.tensor_mul(norm[:], norm[:], ln1w_sb[:])
        nc.vector.tensor_add(norm[:], norm[:], ln1b_sb[:])
        ghat = work.tile([128, GB], BF16, tag="ghat")
        nc.scalar.activation(ghat[:], norm[:], AF.Gelu)

        # ---- fc1 partial over own gene block, AllReduce y1 ----
        # single DMA-engine (XBAR) transpose of ghat into lhsT block layout
        gT = work.tile([128, 4, 128], BF16, tag="gT")
        nc.sync.dma_start_transpose(gT[:], ghat[:])
        ps_y1 = pssm.tile([128, H1], F32, tag="ps_y1")
        for tt in range(4):
            for hh in range(2):
                nc.tensor.matmul(ps_y1[:, ts(hh, 512)], gT[:, tt, :],
                                 w1_sb[:, tt, ts(hh, 512)],
                                 start=(tt == 0), stop=(tt == 3))
        y1p = work.tile([128, H1], BF16, tag="y1p")
        nc.vector.tensor_copy(y1p[:], ps_y1[:])
        cy_in = dram.tile([128, H1], BF16, tag="cy_in")
        nc.sync.dma_start(cy_in[:], y1p[:])
        cy_out = dram.tile([128, H1], BF16, tag="cy_out")
        nc.gpsimd.collective_compute(
            "AllReduce", ALU.add, replica_groups=[list(range(NCORES))],
            ins=[cy_in.opt()], outs=[cy_out.opt()])
        y1h = work.tile([128, H1], BF16, tag="y1h")
        nc.sync.dma_start(y1h[:], cy_out[:, :])
        y1f = work.tile([128, H1], F32, tag="y1f")
        nc.vector.tensor_add(y1f[:], y1h[:], fc1b_sb[:])

        # ---- lnA + gelu + fc2 ----
        y1g = work.tile([128, H1], BF16, tag="y1g")
        _ln_gelu_vec(nc, work, y1f[:], H1, 512,
                     lnAw_sb[:], lnAb_sb[:], y1g[:], "lnA", eps_sb)
        y1T = work.tile([128, 8, 128], BF16, tag="y1T")
        nc.sync.dma_start_transpose(y1T[:], y1g[:])
        ps_y2 = pssm.tile([128, H2], F32, tag="ps_y2")
        for tt in range(8):
            nc.tensor.matmul(ps_y2[:], y1T[:, tt, :], w2_sb[:, tt, :],
                             start=(tt == 0), stop=(tt == 7))
        y2 = work.tile([128, H2], F32, tag="y2")
        nc.vector.tensor_add(y2[:], ps_y2[:], fc2b_sb[:])

        # ---- lnB + gelu + output projection ----
        y2g = work.tile([128, H2], F32, tag="y2g")
        _ln_gelu_vec(nc, work, y2[:], H2, H2, lnBw_sb[:], lnBb_sb[:],
                     y2g[:], "lnB", eps_sb)
        prod = work.tile([128, H2], F32, tag="oprod")
        nc.vector.tensor_mul(prod[:], y2g[:], outw_sb[:])
        red = work.tile([128, 1], F32, tag="ored")
        nc.vector.reduce_sum(red[:], prod[:], axis=mybir.AxisListType.X)
        res = work.tile([128, 1], F32, tag="res")
        nc.vector.tensor_scalar(res[:], red[:], outb_sb[:, 0:1], None, op0=ALU.add)
        nc.sync.dma_start(t["out"][:, :], res[:])

    for _rep in range(repeat):
        emit()

    for p in reversed(ctx_pools):
        p.release()


# ------------------------- host-side preparation -------------------------

def _pm(a):
    """[rows, cols] -> partition-major [128, nch, cols]; rows must be a
    multiple of 128."""
    rows = a.shape[0]
    nch = rows // 128
    return np.ascontiguousarray(
        a.reshape(nch, 128, a.shape[1]).transpose(1, 0, 2))


def _v(x):
    return np.asarray(x, np.float32).reshape(1, -1)


def prepare_in_maps(inputs):
    f = {k: np.asarray(v) for k, v in inputs.items()}
    x = f["x"].astype(np.float32)
    idx = np.asarray(f["impact_indices"]).astype(np.int64)
    mask = np.asarray(f["mask"], np.float32)
    mw = np.asarray(f["mw"], np.float32)

    # sparse-format conversion of the one-nonzero-per-column masked weight
    gene = np.argmax(mask, axis=0)                 # [S] gene of each SNP
    w_eff = mw[gene, np.arange(S)]                 # [S] kept weight values
    order = np.argsort(gene, kind="stable")        # SNPs sorted by gene
    gsort = gene[order]
    core_of = gsort // GC

    # common chunk schedule (SPMD: identical window offsets on all cores):
    # greedy local-gene boundaries s.t. every core has <= 128 SNPs per chunk
    cnt = np.zeros((NCORES, GC), np.int64)
    for c in range(NCORES):
        lg_c = gsort[core_of == c] - c * GC
        cnt[c] = np.bincount(lg_c, minlength=GC)
    assert cnt.max() <= 128, "a single gene exceeds one chunk"
    bounds = []
    g0 = 0
    wmax = 0
    while g0 < GC:
        g1 = g0 + 1
        while g1 < GC and cnt[:, g0:g1 + 1].sum(axis=1).max() <= 128:
            g1 += 1
        bounds.append((g0, g1))
        wmax = max(wmax, g1 - g0)
        g0 = g1
    w = max(16, -(-wmax // 8) * 8)                 # window width, mult of 8
    nch = len(bounds)
    cww = w + B
    offs = tuple(min(a, GB - w) for (a, b) in bounds)
    _CACHE["struct"] = (nch, w, offs)

    x2 = (2.0 * x).astype(np.float32)              # [B, S]

    epw = np.concatenate([
        f["emb"].astype(np.float32).T,
        f["proj_w"].astype(np.float32).T,
        np.stack([f["scale_w"].reshape(-1), f["bias_w"].reshape(-1)],
                 axis=1).astype(np.float32),
    ], axis=1)
    pl4 = np.concatenate([
        np.broadcast_to(_v(f["proj_b"]), (NI, E)),
        np.broadcast_to(_v(f["ln_i_w"]), (NI, E)),
        np.broadcast_to(_v(f["ln_i_b"]), (NI, E)),
        np.broadcast_to(np.array([[f["scale_b"].reshape(()),
                                   f["bias_b"].reshape(())]], np.float32),
                        (NI, 2)),
    ], axis=1).astype(np.float32)
    common = dict(
        epw=np.ascontiguousarray(epw),
        pl4=np.ascontiguousarray(pl4),
        w2A=np.ascontiguousarray(
            f["fc2_w"].astype(BFNP).T.reshape(8, 128, H2)
            .transpose(1, 0, 2)),
    )
    vcommon = {
        "fc1b": f["fc1_b"], "lnAw": f["lnA_w"], "lnAb": f["lnA_b"],
        "fc2b": f["fc2_b"], "lnBw": f["lnB_w"], "lnBb": f["lnB_b"],
        "outw": f["out_w"].reshape(-1), "outb": f["out_b"].reshape(-1),
    }

    fc1_w = f["fc1_w"].astype(np.float32)
    in_maps = []
    for c in range(NCORES):
        ids = order[core_of == c]                  # this core's SNPs
        lg = gsort[core_of == c] - c * GC          # local gene in [0, 500)
        # chunk slices via the common boundaries (lg is sorted)
        lo = np.searchsorted(lg, [a for (a, b) in bounds])
        hi = np.searchsorted(lg, [b for (a, b) in bounds])
        comb = np.zeros((nch, 128, cww), np.float32)
        onep = np.zeros((nch, 128, NI), np.float32)
        for ch in range(nch):
            s0, s1 = lo[ch], hi[ch]
            n = s1 - s0
            rows = np.arange(n)
            comb[ch, rows, lg[s0:s1] - offs[ch]] = w_eff[ids[s0:s1]]
            comb[ch, :n, w:cww] = x2[:, ids[s0:s1]].T
            onep[ch, rows, idx[ids[s0:s1]]] = 1.0
        combA = _pm(comb.reshape(nch * 128, cww).astype(BFNP)) \
            .reshape(128, nch * cww)
        oneA = _pm(onep.reshape(nch * 128, NI).astype(BFNP)) \
            .reshape(128, nch * NI)

        # fc1 weight rows for this core's gene block: [512, H1]
        w1c = np.zeros((GB, H1), np.float32)
        w1c[:GC] = fc1_w[:, c * GC:(c + 1) * GC].T
        w1A = np.ascontiguousarray(
            w1c.astype(BFNP).reshape(4, 128, H1).transpose(1, 0, 2))

        vparts = {"mb": np.zeros(GB, np.float32),
                  "ln1w": np.zeros(GB, np.float32),
                  "ln1b": np.zeros(GB, np.float32)}
        vparts["mb"][:GC] = f["mb"][c * GC:(c + 1) * GC]
        vparts["ln1w"][:GC] = f["ln1_w"][c * GC:(c + 1) * GC]
        vparts["ln1b"][:GC] = f["ln1_b"][c * GC:(c + 1) * GC]
        vparts.update(vcommon)
        vec = np.zeros(VTOT, np.float32)
        for nme, (o, l) in _VOFF.items():
            vec[o:o + l] = np.asarray(vparts[nme], np.float32).reshape(-1)

        m = dict(common)
        m.update(
            combA=combA, oneA=oneA,
            vecs=vec.reshape(1, -1),
            w1A=w1A,
        )
        in_maps.append(m)
    return in_maps


_CACHE = {}
LAST = {}


def kernel(**inputs) -> np.ndarray:
    in_maps = prepare_in_maps(inputs)
    key = ("nc", _CACHE["struct"])
    if key not in _CACHE:
        _CACHE[key] = build_bass(struct=_CACHE["struct"])
    nc = _CACHE[key]
    try:
        res = run_bass_kernel_spmd(nc, in_maps, core_ids=list(range(NCORES)))
    except Exception:
        # transient PJRT-compile/dispatch hiccups have been observed under
        # axon; one retry on a fresh attempt is cheap insurance
        res = run_bass_kernel_spmd(nc, in_maps, core_ids=list(range(NCORES)))
    LAST["results"] = res
    LAST["in_maps"] = in_maps
    return np.asarray(res.results[0]["out"]).reshape(B, 1).astype(np.float32)



# revision 14
# speedup vs baseline: 1.3301x; 1.3301x over previous
"""Trainium2 Bass kernel for nn_AttentionGeneMLP (gnn_message_passing).

Strategy (8 NeuronCores), v3:
  The SNP->gene mask has exactly one nonzero per SNP column, so the masked
  linear is a sparse gather/scatter.  Host-side we convert (mw, mask) from
  dense [G,S] to a sparse block layout (a pure format/layout transform):
    - sort SNPs by their gene, shard SNPs by gene range: core c owns genes
      [500c, 500c+500) and exactly the SNPs mapping to them (~5000).
    - chunk schedule shared by all cores (SPMD): greedy common local-gene
      boundaries such that every core has <= 128 SNPs per chunk; each chunk
      ships a [128, W=16] window tile E holding the masked weight value at
      (snp_row, local_gene - chunk_offset) concatenated with the chunk's x
      columns [128, B].
  Device main loop, engine-balanced:
    - z = x2*sv + bv per chunk via DVE tensor_scalar (4x perf mode; sv/bv
      are per-partition [128,1] scalars),
    - sigma = Sigmoid(z) per mega-tile on the Activation engine,
    - xs = x2*sigma per mega-tile on DVE (bf16, 2x mode),
    - PSUM-windowed matmul accumulation of g on PE; the masked-linear bias
      mb is folded in via a K=1 ones-matmul PSUM preset.
  All gelus are computed as 0.5*x*(1+erf(x/sqrt(2))) with Erf on the
  Activation engine (Erf lives in the same activation table set as Sigmoid,
  so the table is loaded exactly once) and the 0.5 folded into the next
  layer's weights host-side.  Layernorm rstd uses DVE pow(var+eps, -0.5)
  (no Sqrt table).  Identity layernorm affines (w==1, b==0, detected
  host-side and baked into the build key) skip their device ops.
  Bias adds ride K=1 ones-row matmuls (proj_b/scale_b via extra epw rows,
  mb/fc2b via ones-matmul, fc1_b via a constant ghatT pad row /8).
  - ln1 stats: bn_stats partial over the core's 500 genes, AllReduce
    [128,2]; g is PE-transposed into gene-major tiles DURING the AllReduce
    (plus PE warm-up matmuls to hold the tensor-engine p-state), and the
    ln1 normalize+gelu runs in transposed space, feeding fc1 directly as
    lhsT.
  - fc1 sharded by contraction: partial y1 for all H1, AllReduce y1 bf16.
  - lnA/lnB normalize in one 4x tensor_scalar each (identity affine), fc2
    via PE transposes + accumulating matmuls, output projection via DVE
    mul+reduce against a PE-broadcast of [out_w | out_b].

Host-side work is limited to layout: sparse-format conversion, slicing
shards, transposing to the partition-major device layout, dtype casts and
compile-time-constant weight scaling. All model arithmetic runs on device.
"""

import numpy as np
import ml_dtypes

import concourse.bass as bass
import concourse.mybir as mybir
import concourse.tile as tile
from concourse import bacc
from concourse.bass import ts
from concourse.bass_utils import run_bass_kernel_spmd
from concourse.masks import make_identity

F32 = mybir.dt.float32
BF16 = mybir.dt.bfloat16
BFNP = ml_dtypes.bfloat16

# Problem sizes (hardcoded per task contract).
B, S, G, E, NI = 128, 40000, 4000, 16, 4
H1, H2 = 1024, 256
EPS = 1e-5
NCORES = 8
GC = G // NCORES            # 500 genes per core
GB = 512                    # gene block width (500 real + 12 pad)
MEGA = 8                    # s-chunks per DMA mega-tile
R2I = float(1.0 / np.sqrt(2.0))
WARM = 8                    # PE p-state warm-up matmuls per collective wait

# bf16 parameter bundle: rows usable pre-broadcast ([1,N]) or as full
# partition-broadcast tiles ([128,N]).  offsets into vbf [1, VBT]
_VSPECS = [("mb", GB), ("fc2b", H2), ("fc1b", H1), ("lnAw", H1), ("lnAb", H1),
           ("lnBw", H2), ("lnBb", H2), ("outw", H2), ("outb", 1)]
_VOFF = {}
_o = 0
for _n, _l in _VSPECS:
    _VOFF[_n] = (_o, _l)
    _o += _l
VBT = _o
AF = mybir.ActivationFunctionType
ALU = mybir.AluOpType


def _mega_starts(nch):
    starts = []
    c = 0
    while c < nch:
        starts.append((c, min(MEGA, nch - c)))
        c += MEGA
    return starts


def _rsqrt(nc, work, out, var_ap, tag, iters=2):
    """out[P,1] = (var+EPS)^-0.5 on DVE only (quake seed + Newton)."""
    I32 = mybir.dt.int32
    ve = work.tile(list(var_ap.shape), F32, tag=f"{tag}_ve")
    nc.vector.tensor_scalar(ve[:], var_ap, EPS, None, op0=ALU.add)
    vh = work.tile(list(var_ap.shape), F32, tag=f"{tag}_vh")
    nc.vector.tensor_scalar(vh[:], ve[:], -0.5, None, op0=ALU.mult)
    yi = out.bitcast(I32)
    nc.vector.tensor_scalar(yi, ve[:].bitcast(I32), 1, None,
                            op0=ALU.logical_shift_right)
    nc.vector.tensor_scalar(yi, yi, -1, None, op0=ALU.bitwise_xor)
    nc.vector.tensor_scalar(yi, yi, 0x5f3759e0, None, op0=ALU.add)
    t = work.tile(list(var_ap.shape), F32, tag=f"{tag}_t")
    for _ in range(iters):
        nc.vector.scalar_tensor_tensor(t[:], in0=out, scalar=vh[:, 0:1],
                                       in1=out, op0=ALU.mult, op1=ALU.mult)
        nc.vector.scalar_tensor_tensor(out, in0=t[:], scalar=1.5,
                                       in1=out, op0=ALU.add, op1=ALU.mult)


def build_bass(repeat=1, struct=None):
    """Build + compile the 8-core SPMD Bass module. Returns nc."""
    if struct is None:
        struct = _CACHE["struct"]
    nch, w, offs, flags = struct
    cww = w + B
    nc = bacc.Bacc("TRN2", target_bir_lowering=False, debug=False,
                   num_devices=NCORES)

    def din(name, shape, dt):
        return nc.dram_tensor(name, shape, dt, kind="ExternalInput")

    # big stream (partition-major: [p, chunk, E|x2] flattened on last dims)
    combA = din("combA", [128, nch * cww], BF16)
    # attention path: one-hot planes transposed [NI, nch*128]
    oneT = din("oneT", [NI, nch * 128], BF16)
    # packed tiny attention params; rows 0..E-1 = [embT|projwT|0.5*swbw],
    # row E = [ones | proj_b | scale_b bias_b] (bias fold via K=E+1 matmul)
    epw = din("epw", [E + 1, NI + E + 2], F32)
    pl4 = din("pl4", [NI, 2 * E], F32)
    # per-feature bf16 rows (mb, fc2b used as [1,N]; rest broadcast)
    vbf = din("vbf", [1, VBT], BF16)
    # ln1 w|b in gene-tile-major per-partition layout [128, 4+4]
    lntb = din("lntb", [128, 8], F32)
    w1A = din("w1A", [128, 4, H1], BF16)
    w2A = din("w2A", [128, 8, H2], BF16)

    out = nc.dram_tensor("out", [B, 1], F32, kind="ExternalOutput")

    tensors = {k: v for k, v in locals().items()}
    with tile.TileContext(nc) as tc:
        _body(tc, tensors, struct, repeat)
    nc.compile()
    return nc


def _body(tc, t, struct, repeat=1):
    nch, w, offs, flags = struct
    iden_i, iden1, idenA, idenB = flags
    cww = w + B
    nc = tc.nc
    ctx_pools = []

    def pool(**kw):
        p = tc.alloc_tile_pool(**kw)
        ctx_pools.append(p)
        return p

    const = pool(name="const", bufs=1)
    work = pool(name="work", bufs=1)
    combp = pool(name="combp", bufs=6)
    zp = pool(name="zp", bufs=6)
    xsp = pool(name="xsp", bufs=6)
    psg = pool(name="psg", bufs=1, space="PSUM")
    pssm = pool(name="pssm", bufs=1, space="PSUM")
    dram = pool(name="dram", bufs=1, space="DRAM")

    def emit():
        # ---- tiny constants first (keep the comb stream unblocked) ----
        eps_sb = const.tile([128, 1], F32, tag="eps")
        nc.vector.memset(eps_sb[:], EPS)
        # prime the sigmoid/erf activation table ASAP (overlaps DMAs)
        dum = work.tile([1, 1], F32, tag="dum")
        nc.scalar.activation(dum[:], eps_sb[0:1, 0:1], AF.Sigmoid)

        epw_sb = const.tile([128, NI + E + 2], F32, tag="epw")
        nc.vector.memset(epw_sb[:], 0.0)
        nc.sync.dma_start(epw_sb[:E + 1, :], t["epw"][:, :])
        embT_sb = epw_sb[:, 0:NI]
        projwT_sb = epw_sb[:, NI:NI + E]
        swbw_sb = epw_sb[:, NI + E:NI + E + 2]
        if not iden_i:
            pl4_sb = const.tile([NI, 2 * E], F32, tag="c_pl4")
            nc.gpsimd.dma_start(pl4_sb[:], t["pl4"][:, :])
        oneT_sb = const.tile([NI, nch * 128], BF16, tag="c_oneT")
        nc.gpsimd.dma_start(oneT_sb[:], t["oneT"][:, :])
        vbf_sb = const.tile([128, VBT], BF16, tag="b_vbf")
        nc.gpsimd.dma_start(vbf_sb[0:1, :], t["vbf"][:, :])

        def vrow(name):
            o, l = _VOFF[name]
            return vbf_sb[0:1, o:o + l]

        def vtile(name):
            o, l = _VOFF[name]
            return vbf_sb[:, o:o + l]

        ident_f = const.tile([128, 128], F32, tag="ident_f")
        make_identity(nc, ident_f[:])
        ident_b = const.tile([128, 128], BF16, tag="ident_b")
        make_identity(nc, ident_b[:])
        ones_b = const.tile([1, 128], BF16, tag="ones_b")
        nc.vector.memset(ones_b[:], 1.0)

        # ---- attention scale/bias tables (tiny, K padded to 128) ----
        # h4 = emb @ proj_w.T + proj_b (ones-row in epw carries proj_b)
        ps_h4 = pssm.tile([128, 128], F32, tag="ps_small", name="ps_h4")[:NI, :E]
        nc.tensor.matmul(ps_h4[:], embT_sb[:], projwT_sb[:], start=True, stop=True)
        h4 = work.tile([NI, E], F32, tag="h4")
        # ln over E (free dim), partitions = NI
        st4 = work.tile([NI, 6], F32, tag="st4")
        nc.vector.bn_stats(out=st4[:], in_=ps_h4[:])
        mv4 = work.tile([NI, 2], F32, tag="mv4")
        nc.vector.bn_aggr(out=mv4[:], in_=st4[:])
        rstd4 = work.tile([NI, 1], F32, tag="rstd4")
        _rsqrt(nc, work, rstd4[:], mv4[:, 1:2], "r4", iters=3)
        nc.vector.tensor_scalar(h4[:], ps_h4[:], mv4[:, 0:1], rstd4[:, 0:1],
                                op0=ALU.subtract, op1=ALU.mult)
        if not iden_i:
            nc.vector.tensor_mul(h4[:], h4[:], pl4_sb[:, 0:E])
            nc.vector.tensor_add(h4[:], h4[:], pl4_sb[:, E:2 * E])
        # gelu via erf: h4g' = (erf(h4/sqrt2)+1)*h4  (0.5 folded into swbw)
        e4 = work.tile([NI, E], F32, tag="e4")
        nc.scalar.activation(e4[:], h4[:], AF.Erf, scale=R2I)
        h4g = work.tile([128, E + 1], F32, tag="h4g")
        nc.vector.memset(h4g[:], 0.0)
        nc.vector.scalar_tensor_tensor(h4g[:NI, 0:E], in0=e4[:], scalar=1.0,
                                       in1=h4[:], op0=ALU.add, op1=ALU.mult)
        # ones column -> transposes into row E (rides scale_b/bias_b of epw)
        nc.vector.memset(h4g[0:NI, E:E + 1], 1.0)
        ps_t4 = pssm.tile([128, 128], F32, tag="ps_small",
                          name="ps_t4")[:E + 1, :]
        nc.tensor.transpose(ps_t4[:], h4g[:], ident_f[:])
        h4gT = work.tile([128, NI], F32, tag="h4gT")
        nc.vector.memset(h4gT[:], 0.0)
        nc.vector.tensor_copy(h4gT[:E + 1, :], ps_t4[:, :NI])
        ps_tab = pssm.tile([128, 128], F32, tag="ps_small", name="ps_tab")[:NI, :2]
        nc.tensor.matmul(ps_tab[:], h4gT[:], swbw_sb[:], start=True, stop=True)
        tab = work.tile([NI, 2], BF16, tag="tab")
        nc.scalar.activation(tab[:], ps_tab[:], AF.Copy)

        # per-chunk per-SNP [scale|bias] via K=NI PE contractions against
        # the host one-hot planes: svbv[:, 2c:2c+2] = onehot_c @ tab
        svbv_ps = pssm.tile([128, 2 * nch], F32, tag="ps_svbv")
        for c in range(nch):
            nc.tensor.matmul(svbv_ps[:, 2 * c:2 * c + 2],
                             oneT_sb[:, ts(c, 128)], tab[:],
                             start=True, stop=True)
        # SBUF copy so both DVE and Pool can read the scalars
        svbv = const.tile([128, 2 * nch], F32, tag="svbv")
        nc.scalar.activation(svbv[:], svbv_ps[:], AF.Copy)

        # ---- main loop: stream [E|x2] chunks, accumulate g in PSUM ----
        g_ps = psg.tile([128, GB], F32, tag="g_ps")
        # PSUM preset = mb broadcast over batch rows (K=1 ones-matmul)
        nc.tensor.matmul(g_ps[:], ones_b[:], vrow("mb"),
                         start=True, stop=False, skip_group_check=True)
        combA = t["combA"]
        for (c0, k) in _mega_starts(nch):
            comb = combp.tile([128, k, cww], BF16, tag="comb")
            nc.sync.dma_start(comb[:], combA[:, c0 * cww:(c0 + k) * cww]
                              .rearrange("p (k n) -> p k n", k=k))
            xv = comb[:, :, w:cww]                      # [128, k, B]
            zt = zp.tile([128, k, B], BF16, tag="zt")
            for j in range(k):
                c = c0 + j
                eng = nc.vector if j % 2 == 0 else nc.gpsimd
                eng.tensor_scalar(zt[:, j, :], xv[:, j, :],
                                  svbv[:, 2 * c:2 * c + 1],
                                  svbv[:, 2 * c + 1:2 * c + 2],
                                  op0=ALU.mult, op1=ALU.add)
            xs = xsp.tile([128, k, B], BF16, tag="xs")
            for h0 in range(0, k, 4):
                h1 = min(h0 + 4, k)
                nc.scalar.activation(zt[:, h0:h1, :], zt[:, h0:h1, :],
                                     AF.Sigmoid)
                nc.vector.tensor_mul(xs[:, h0:h1, :], xv[:, h0:h1, :],
                                     zt[:, h0:h1, :])
                for j in range(h0, h1):
                    c = c0 + j
                    nc.tensor.matmul(g_ps[:, offs[c]:offs[c] + w],
                                     xs[:, j, :], comb[:, j, 0:w],
                                     start=False, stop=(c == nch - 1),
                                     skip_group_check=True)

        # heavy weight loads ride behind the comb stream (needed post-loop)
        w1_sb = const.tile([128, 4, H1], BF16, tag="c_w1A")
        nc.sync.dma_start(w1_sb[:], t["w1A"][:, :, :])
        w2_sb = const.tile([128, 8, H2], BF16, tag="c_w2A")
        nc.sync.dma_start(w2_sb[:], t["w2A"][:, :, :])
        if not iden1:
            lntb_sb = const.tile([128, 8], F32, tag="c_lntb")
            nc.sync.dma_start(lntb_sb[:], t["lntb"][:, :])

        # ---- ln1 partial stats over the core's 500 real genes ----
        st1 = work.tile([128, 6], F32, tag="st1")
        nc.vector.bn_stats(out=st1[:], in_=g_ps[:, 0:GC])
        mv1 = work.tile([128, 2], F32, tag="mv1")
        nc.vector.bn_aggr(out=mv1[:], in_=st1[:])
        # convert to cross-core sums: s1 = GC*mean ; s2 = GC*(var + mean^2)
        cs = work.tile([128, 2], F32, tag="cs")
        msq1 = work.tile([128, 1], F32, tag="msq1")
        nc.vector.tensor_mul(msq1[:], mv1[:, 0:1], mv1[:, 0:1])
        nc.vector.tensor_scalar(cs[:, 0:1], mv1[:, 0:1], float(GC), None,
                                op0=ALU.mult)
        nc.vector.tensor_scalar(cs[:, 1:2], msq1[:], mv1[:, 1:2], float(GC),
                                op0=ALU.add, op1=ALU.mult)
        cs_in = dram.tile([128, 2], F32, tag="cs_in")
        nc.sync.dma_start(cs_in[:], cs[:])
        cs_out = dram.tile([128, 2], F32, tag="cs_out")
        nc.gpsimd.collective_compute(
            "AllReduce", ALU.add, replica_groups=[list(range(NCORES))],
            ins=[cs_in.opt()], outs=[cs_out.opt()])

        # ---- overlapped with AllReduce#1: transpose g to gene-major ----
        g_sb = work.tile([128, GB], BF16, tag="g_sb")
        nc.vector.tensor_copy(g_sb[:], g_ps[:])
        gT = work.tile([128, 4, 128], BF16, tag="gT")
        for tt in range(4):
            ps_tr = pssm.tile([128, 128], BF16, tag=f"ps_tr{tt % 2}",
                              name=f"tr{tt}")
            nc.tensor.transpose(ps_tr[:], g_sb[:, ts(tt, 128)], ident_b[:])
            nc.scalar.activation(gT[:, tt, :], ps_tr[:], AF.Copy)
        if not idenA:
            for nm in ("lnAw", "lnAb"):
                o, l = _VOFF[nm]
                nc.gpsimd.partition_broadcast(vbf_sb[:, o:o + l],
                                              vbf_sb[0:1, o:o + l])
        if not idenB:
            for nm in ("lnBw", "lnBb"):
                o, l = _VOFF[nm]
                nc.gpsimd.partition_broadcast(vbf_sb[:, o:o + l],
                                              vbf_sb[0:1, o:o + l])

        ssum = work.tile([128, 2], F32, tag="ssum")
        nc.sync.dma_start(ssum[:], cs_out[:, :])

        # ---- ln1 normalization in transposed (gene-major) space ----
        mv = work.tile([128, 2], F32, tag="ln1_mv")
        nc.vector.tensor_scalar(mv[:], ssum[:], 1.0 / G, None, op0=ALU.mult)
        var1 = work.tile([128, 1], F32, tag="ln1_var")
        nc.vector.tensor_mul(var1[:], mv[:, 0:1], mv[:, 0:1])
        nc.vector.tensor_sub(var1[:], mv[:, 1:2], var1[:])
        pack2 = work.tile([128, 2], F32, tag="pack2")
        _rsqrt(nc, work, pack2[:, 1:2], var1[:], "r1")         # rstd
        nc.vector.tensor_mul(pack2[:, 0:1], mv[:, 0:1], pack2[:, 1:2])  # m*rstd
        # broadcast rows [mr | rstd] over all partitions via PE
        ps_mv = pssm.tile([128, 128], F32, tag="ps_tr0", name="ps_mv")
        nc.tensor.transpose(ps_mv[:1, :], pack2[:, 0:1], ident_f[:])
        ps_mv2 = pssm.tile([128, 128], F32, tag="ps_tr1", name="ps_mv2")
        nc.tensor.transpose(ps_mv2[:1, :], pack2[:, 1:2], ident_f[:])
        row2 = work.tile([1, 256], BF16, tag="row2")
        nc.scalar.activation(row2[:, 0:128], ps_mv[:1, :], AF.Copy)
        nc.scalar.activation(row2[:, 128:256], ps_mv2[:1, :], AF.Copy)
        ps_bc = pssm.tile([128, 256], F32, tag="ps_small", name="ps_bc")
        nc.tensor.matmul(ps_bc[:], ones_b[:], row2[:], start=True, stop=True)
        bc = work.tile([128, 256], BF16, tag="bc")
        nc.scalar.activation(bc[:], ps_bc[:], AF.Copy)
        mrB = bc[:, 0:128].unsqueeze(1).broadcast_to([128, 4, 128])
        rstdB = bc[:, 128:256].unsqueeze(1).broadcast_to([128, 4, 128])
        argT = work.tile([128, 4, 128], BF16, tag="argT")
        nc.vector.tensor_mul(argT[:], gT[:], rstdB)
        nc.vector.tensor_sub(argT[:], argT[:], mrB)
        if not iden1:
            for tt in range(4):
                nc.vector.tensor_scalar(argT[:, tt, :], argT[:, tt, :],
                                        lntb_sb[:, tt:tt + 1],
                                        lntb_sb[:, 4 + tt:5 + tt],
                                        op0=ALU.mult, op1=ALU.add)
        eT = work.tile([128, 4, 128], BF16, tag="eT")
        nc.scalar.activation(eT[:], argT[:], AF.Erf, scale=R2I)
        ghatT = work.tile([128, 4, 128], BF16, tag="ghatT")
        nc.vector.scalar_tensor_tensor(ghatT[:], in0=eT[:], scalar=1.0,
                                       in1=argT[:], op0=ALU.add, op1=ALU.mult)

        # ---- fc1 partial over own gene block, AllReduce y1 ----
        # PSUM preset carries fc1_b/8 through the AllReduce
        ps_y1 = pssm.tile([128, H1], F32, tag="ps_y1")
        fo = _VOFF["fc1b"][0]
        for hh in range(2):
            nc.tensor.matmul(ps_y1[:, ts(hh, 512)], ones_b[:],
                             vbf_sb[0:1, fo + 512 * hh:fo + 512 * hh + 512],
                             start=True, stop=False, skip_group_check=True)
        for tt in range(4):
            for hh in range(2):
                nc.tensor.matmul(ps_y1[:, ts(hh, 512)], ghatT[:, tt, :],
                                 w1_sb[:, tt, ts(hh, 512)],
                                 start=False, stop=(tt == 3),
                                 skip_group_check=True)
        y1p = work.tile([128, H1], BF16, tag="y1p")
        nc.scalar.activation(y1p[:], ps_y1[:], AF.Copy)
        cy_in = dram.tile([128, H1], BF16, tag="cy_in")
        nc.sync.dma_start(cy_in[:], y1p[:])
        cy_out = dram.tile([128, H1], BF16, tag="cy_out")
        nc.gpsimd.collective_compute(
            "AllReduce", ALU.add, replica_groups=[list(range(NCORES))],
            ins=[cy_in.opt()], outs=[cy_out.opt()])
        # overlapped with AllReduce#2: broadcast [out_w|out_b], warm PE
        psB = pssm.tile([128, H2 + 1], F32, tag="ps_y1", name="psB")
        nc.tensor.matmul(psB[:], ones_b[:], vbf_sb[0:1, _VOFF["outw"][0]:
                                                    _VOFF["outw"][0] + H2 + 1],
                         start=True, stop=True)
        # stage [out_w | out_b] into SBUF while AllReduce#2 is in flight
        outwb = work.tile([128, H2 + 1], BF16, tag="outwb")
        nc.scalar.activation(outwb[:], psB[:], AF.Copy)
        y1h = work.tile([128, H1], BF16, tag="y1h")
        nc.sync.dma_start(y1h[:], cy_out[:, :])

        # ---- lnA + gelu(erf) in transposed space ----
        # stats on the row-space y1h (DVE) overlap the PE transpose
        stA = work.tile([128, 2, 6], F32, tag="stA")
        y1h2 = y1h.rearrange("p (a b) -> p a b", b=512)
        for i in range(2):
            nc.vector.bn_stats(out=stA[:, i, :], in_=y1h2[:, i, :])
        mvA = work.tile([128, 2], F32, tag="mvA")
        nc.vector.bn_aggr(out=mvA[:], in_=stA[:])
        packA = work.tile([128, 2], F32, tag="packA")
        _rsqrt(nc, work, packA[:, 1:2], mvA[:, 1:2], "rA")     # rstd
        nc.vector.tensor_mul(packA[:, 0:1], mvA[:, 0:1], packA[:, 1:2])
        if idenA:
            trsrcA = y1h
        else:
            argA = work.tile([128, H1], BF16, tag="argA")
            nc.vector.tensor_scalar(argA[:], y1h[:], mvA[:, 0:1],
                                    packA[:, 1:2],
                                    op0=ALU.subtract, op1=ALU.mult)
            nc.vector.tensor_mul(argA[:], argA[:], vtile("lnAw"))
            nc.vector.tensor_add(argA[:], argA[:], vtile("lnAb"))
            trsrcA = argA
        y1hT = work.tile([128, 8, 128], BF16, tag="y1hT")
        for tt in range(8):
            ps_tr = pssm.tile([128, 128], BF16, tag=f"ps_tr{tt % 2}",
                              name=f"trA{tt}")
            nc.tensor.transpose(ps_tr[:], trsrcA[:, ts(tt, 128)], ident_b[:])
            if tt % 2 == 0:
                nc.scalar.activation(y1hT[:, tt, :], ps_tr[:], AF.Copy)
            else:
                nc.vector.tensor_copy(y1hT[:, tt, :], ps_tr[:])
        argAT = work.tile([128, 8, 128], BF16, tag="argAT")
        if idenA:
            ps_mvA = pssm.tile([128, 128], F32, tag="ps_tr0", name="ps_mvA")
            nc.tensor.transpose(ps_mvA[:1, :], packA[:, 0:1], ident_f[:])
            ps_mvA2 = pssm.tile([128, 128], F32, tag="ps_tr1", name="ps_mvA2")
            nc.tensor.transpose(ps_mvA2[:1, :], packA[:, 1:2], ident_f[:])
            rowA = work.tile([1, 256], BF16, tag="rowA")
            nc.scalar.activation(rowA[:, 0:128], ps_mvA[:1, :], AF.Copy)
            nc.scalar.activation(rowA[:, 128:256], ps_mvA2[:1, :], AF.Copy)
            ps_bcA = pssm.tile([128, 256], F32, tag="ps_small", name="ps_bcA")
            nc.tensor.matmul(ps_bcA[:], ones_b[:], rowA[:], start=True, stop=True)
            bcA = work.tile([128, 256], BF16, tag="bcA")
            nc.scalar.activation(bcA[:], ps_bcA[:], AF.Copy)
            eA = work.tile([128, 8, 128], BF16, tag="eA")
            y1gT = work.tile([128, 8, 128], BF16, tag="y1gT")
            for hh in range(2):
                sl = slice(4 * hh, 4 * hh + 4)
                nc.vector.tensor_mul(argAT[:, sl, :], y1hT[:, sl, :],
                                     bcA[:, 128:256].unsqueeze(1)
                                     .broadcast_to([128, 4, 128]))
                nc.vector.tensor_sub(argAT[:, sl, :], argAT[:, sl, :],
                                     bcA[:, 0:128].unsqueeze(1)
                                     .broadcast_to([128, 4, 128]))
                nc.scalar.activation(eA[:, sl, :], argAT[:, sl, :],
                                     AF.Erf, scale=R2I)
                nc.vector.scalar_tensor_tensor(y1gT[:, sl, :],
                                               in0=eA[:, sl, :], scalar=1.0,
                                               in1=argAT[:, sl, :],
                                               op0=ALU.add, op1=ALU.mult)
        else:
            eA = work.tile([128, 8, 128], BF16, tag="eA")
            nc.scalar.activation(eA[:], y1hT[:], AF.Erf, scale=R2I)
            y1gT = work.tile([128, 8, 128], BF16, tag="y1gT")
            nc.vector.scalar_tensor_tensor(y1gT[:], in0=eA[:], scalar=1.0,
                                           in1=y1hT[:],
                                           op0=ALU.add, op1=ALU.mult)
        ps_y2 = pssm.tile([128, H2], F32, tag="ps_y2")
        nc.tensor.matmul(ps_y2[:], ones_b[:], vrow("fc2b"),
                         start=True, stop=False)
        for tt in range(8):
            nc.tensor.matmul(ps_y2[:], y1gT[:, tt, :], w2_sb[:, tt, :],
                             start=False, stop=(tt == 7))

        # ---- lnB + gelu(erf) + output projection ----
        stB = work.tile([128, 6], F32, tag="stB")
        nc.vector.bn_stats(out=stB[:], in_=ps_y2[:])
        mvB = work.tile([128, 2], F32, tag="mvB")
        nc.vector.bn_aggr(out=mvB[:], in_=stB[:])
        rstdB2 = work.tile([128, 1], F32, tag="rstdB2")
        _rsqrt(nc, work, rstdB2[:], mvB[:, 1:2], "rB")
        argB = work.tile([128, H2], BF16, tag="argB")
        nc.vector.tensor_scalar(argB[:], ps_y2[:], mvB[:, 0:1], rstdB2[:, 0:1],
                                op0=ALU.subtract, op1=ALU.mult)
        if not idenB:
            nc.vector.tensor_mul(argB[:], argB[:], vtile("lnBw"))
            nc.vector.tensor_add(argB[:], argB[:], vtile("lnBb"))
        eB = work.tile([128, H2], BF16, tag="eB")
        nc.scalar.activation(eB[:], argB[:], AF.Erf, scale=R2I)
        y2g = work.tile([128, H2], BF16, tag="y2g")
        nc.vector.scalar_tensor_tensor(y2g[:], in0=eB[:], scalar=1.0,
                                       in1=argB[:], op0=ALU.add, op1=ALU.mult)
        prod = work.tile([128, H2], BF16, tag="oprod")
        nc.vector.tensor_mul(prod[:], y2g[:], outwb[:, 0:H2])
        red = work.tile([128, 1], F32, tag="ored")
        nc.vector.reduce_sum(red[:], prod[:], axis=mybir.AxisListType.X)
        res = work.tile([128, 1], F32, tag="res")
        nc.vector.tensor_add(res[:], red[:], outwb[:, H2:H2 + 1])
        nc.sync.dma_start(t["out"][:, :], res[:])

    for _rep in range(repeat):
        emit()

    for p in reversed(ctx_pools):
        p.release()


# ------------------------- host-side preparation -------------------------

def _pm(a):
    """[rows, cols] -> partition-major [128, nch, cols]; rows must be a
    multiple of 128."""
    rows = a.shape[0]
    nch = rows // 128
    return np.ascontiguousarray(
        a.reshape(nch, 128, a.shape[1]).transpose(1, 0, 2))


def _iden(wv, bv):
    return bool(np.all(np.asarray(wv) == 1.0) and np.all(np.asarray(bv) == 0.0))


def prepare_in_maps(inputs):
    f = {k: np.asarray(v) for k, v in inputs.items()}
    x = f["x"].astype(np.float32)
    idx = np.asarray(f["impact_indices"]).astype(np.int64)
    mask = np.asarray(f["mask"], np.float32)
    mw = np.asarray(f["mw"], np.float32)

    # sparse-format conversion of the one-nonzero-per-column masked weight
    gene = np.argmax(mask, axis=0)                 # [S] gene of each SNP
    w_eff = mw[gene, np.arange(S)]                 # [S] kept weight values
    order = np.argsort(gene, kind="stable")        # SNPs sorted by gene
    gsort = gene[order]
    core_of = gsort // GC

    # common chunk schedule (SPMD: identical window offsets on all cores)
    cnt = np.zeros((NCORES, GC), np.int64)
    for c in range(NCORES):
        lg_c = gsort[core_of == c] - c * GC
        cnt[c] = np.bincount(lg_c, minlength=GC)
    assert cnt.max() <= 128, "a single gene exceeds one chunk"
    bounds = []
    g0 = 0
    wmax = 0
    while g0 < GC:
        g1 = g0 + 1
        while g1 < GC and cnt[:, g0:g1 + 1].sum(axis=1).max() <= 128:
            g1 += 1
        bounds.append((g0, g1))
        wmax = max(wmax, g1 - g0)
        g0 = g1
    w = max(16, -(-wmax // 8) * 8)                 # window width, mult of 8
    nch = len(bounds)
    cww = w + B
    offs = tuple(min(a, GB - w) for (a, b) in bounds)

    flags = (_iden(f["ln_i_w"], f["ln_i_b"]),
             _iden(f["ln1_w"], f["ln1_b"]),
             _iden(f["lnA_w"], f["lnA_b"]),
             _iden(f["lnB_w"], f["lnB_b"]))
    _CACHE["struct"] = (nch, w, offs, flags)

    # attn's *2 is folded into the E weights (g = sum (x*sigma)*(2w))
    x1 = x.astype(np.float32)                      # [B, S]

    # rows 0..E-1: [embT | projwT | 0.5*swbw]; row E: [1s | proj_b | sb bb]
    epw = np.zeros((E + 1, NI + E + 2), np.float32)
    epw[:E, 0:NI] = f["emb"].astype(np.float32).T
    epw[:E, NI:NI + E] = f["proj_w"].astype(np.float32).T
    epw[:E, NI + E:] = 0.5 * np.stack(
        [f["scale_w"].reshape(-1), f["bias_w"].reshape(-1)], axis=1)
    epw[E, 0:NI] = 1.0
    epw[E, NI:NI + E] = np.asarray(f["proj_b"], np.float32)
    epw[E, NI + E:] = [float(np.reshape(f["scale_b"], ())),
                       float(np.reshape(f["bias_b"], ()))]

    def _v(z):
        return np.asarray(z, np.float32).reshape(1, -1)

    pl4 = np.concatenate([
        np.broadcast_to(_v(f["ln_i_w"]), (NI, E)),
        np.broadcast_to(_v(f["ln_i_b"]), (NI, E)),
    ], axis=1).astype(np.float32)
    common = dict(
        epw=np.ascontiguousarray(epw),
        pl4=np.ascontiguousarray(pl4),
        # lnA gelu's 0.5 folded into fc2 weights
        w2A=np.ascontiguousarray(
            (0.5 * f["fc2_w"].astype(np.float32)).astype(BFNP)
            .T.reshape(8, 128, H2).transpose(1, 0, 2)),
    )
    vcommon = {
        "fc2b": f["fc2_b"], "fc1b": np.asarray(f["fc1_b"]) / NCORES,
        "lnAw": f["lnA_w"], "lnAb": f["lnA_b"],
        "lnBw": f["lnB_w"], "lnBb": f["lnB_b"],
        # lnB gelu's 0.5 folded into the output projection
        "outw": 0.5 * np.asarray(f["out_w"], np.float32).reshape(-1),
        "outb": f["out_b"].reshape(-1),
    }

    fc1_w = f["fc1_w"].astype(np.float32)
    in_maps = []
    for c in range(NCORES):
        ids = order[core_of == c]                  # this core's SNPs
        lg = gsort[core_of == c] - c * GC          # local gene in [0, 500)
        lo = np.searchsorted(lg, [a for (a, b) in bounds])
        hi = np.searchsorted(lg, [b for (a, b) in bounds])
        comb = np.zeros((nch, 128, cww), np.float32)
        onep = np.zeros((nch, 128, NI), np.float32)
        for ch in range(nch):
            s0, s1 = lo[ch], hi[ch]
            n = s1 - s0
            rows = np.arange(n)
            comb[ch, rows, lg[s0:s1] - offs[ch]] = 2.0 * w_eff[ids[s0:s1]]
            comb[ch, :n, w:cww] = x1[:, ids[s0:s1]].T
            onep[ch, rows, idx[ids[s0:s1]]] = 1.0
        combA = _pm(comb.reshape(nch * 128, cww).astype(BFNP)) \
            .reshape(128, nch * cww)
        oneT = np.ascontiguousarray(
            onep.transpose(2, 0, 1).reshape(NI, nch * 128).astype(BFNP))

        # fc1 weight rows for this core's gene block: [512, H1]
        # (0.5 erf-gelu fold on real rows; row 500 carries fc1_b/8)
        w1c = np.zeros((GB, H1), np.float32)
        w1c[:GC] = 0.5 * fc1_w[:, c * GC:(c + 1) * GC].T
        w1A = np.ascontiguousarray(
            w1c.astype(BFNP).reshape(4, 128, H1).transpose(1, 0, 2))

        # ln1 w|b in gene-tile-major per-partition layout [128, 4+4]
        wpad = np.zeros(GB, np.float32)
        bpad = np.zeros(GB, np.float32)
        wpad[:GC] = f["ln1_w"][c * GC:(c + 1) * GC]
        bpad[:GC] = f["ln1_b"][c * GC:(c + 1) * GC]
        lntb = np.concatenate([wpad.reshape(4, 128).T,
                               bpad.reshape(4, 128).T], axis=1)

        mbpad = np.zeros(GB, np.float32)
        mbpad[:GC] = f["mb"][c * GC:(c + 1) * GC]
        vparts = {"mb": mbpad}
        vparts.update(vcommon)
        vec = np.zeros(VBT, np.float32)
        for nme, (o, l) in _VOFF.items():
            vec[o:o + l] = np.asarray(vparts[nme], np.float32).reshape(-1)

        m = dict(common)
        m.update(
            combA=combA, oneT=oneT,
            vbf=vec.reshape(1, -1).astype(BFNP),
            lntb=np.ascontiguousarray(lntb, np.float32),
            w1A=w1A,
        )
        in_maps.append(m)
    return in_maps


_CACHE = {}
LAST = {}


def kernel(**inputs) -> np.ndarray:
    in_maps = prepare_in_maps(inputs)
    key = ("nc", _CACHE["struct"])
    if key not in _CACHE:
        _CACHE[key] = build_bass(struct=_CACHE["struct"])
    nc = _CACHE[key]
    try:
        res = run_bass_kernel_spmd(nc, in_maps, core_ids=list(range(NCORES)))
    except Exception:
        # transient PJRT-compile/dispatch hiccups have been observed under
        # axon; one retry on a fresh attempt is cheap insurance
        res = run_bass_kernel_spmd(nc, in_maps, core_ids=list(range(NCORES)))
    LAST["results"] = res
    LAST["in_maps"] = in_maps
    return np.asarray(res.results[0]["out"]).reshape(B, 1).astype(np.float32)
